# revision 1
# baseline (speedup 1.0000x reference)
"""Multi-head self-attention Trainium2 kernel.

Problem: x[2, 2048, 768] -> MHSA (12 heads, head_dim 64) -> out[2, 2048, 768].

Sharding over 8 NeuronCores: core c handles batch c//4 and heads
[3*(c%4), 3*(c%4)+3). Each core computes its 3 heads' attention and a
row-split partial of the output projection over its 192 channels; the host
sums the 4 partials per batch and transposes.

On-chip layouts are transposed (channels on partitions) so softmax row-sums
ride the attn@V matmul as an appended ones-column and exp is a single
ScalarE pass over PSUM scores (no max-subtraction: logits ~ N(0,1)).
Matmul operands are bf16 (fp32 PSUM accumulation); softmax normalization is
fp32. Chunk-0 scores are emitted before the V projection so ScalarE starts
early while V-matmuls fill TensorE gaps. The output projection contracts
h0/h1 as one K=128 matmul (weights stacked on partitions) plus a K=64 h2
matmul; partials leave as bf16 and the host sums them in fp32. Dummy
matmuls on a zeroed tile bridge the initial DMA phase and the pre-epilogue
divide chain so the PE HAM clock gate stays at K=8/8; input DMAs issue
from the Pool sequencer (~25ns each vs ~660ns on SP).
"""

import sys

sys.path.insert(0, "/opt/trn_rl_repo")

import numpy as np

EMBED = 768
N_SEQ = 2048
N_HEADS_CORE = 3
HD = 64
N_CORES = 8
KT = EMBED // 128  # 6 contraction tiles for the projections
MT = N_SEQ // 128  # 16 seq / key-row tiles
QCH = 512  # q-chunk (one fp32 PSUM bank)
NCH = N_SEQ // QCH  # 4 chunks
NG = MT // 2  # 8 groups of 2 krow-tiles

# fast-exp2 constants: bf16 bits = round(s*0.125*log2e*128 + 127*128 + adj)
FEXP_C1 = 0.125 * 1.4426950408889634 * 128.0
FEXP_C2 = 127.0 * 128.0 + 0.5 - 5.5

_CACHED = {}


def _build(stages=99):
    from concourse import bacc
    import concourse.tile as tile
    import concourse.mybir as mybir

    F32 = mybir.dt.float32
    BF16 = mybir.dt.bfloat16
    I16 = mybir.dt.int16
    EXP = mybir.ActivationFunctionType.Exp
    MULT = mybir.AluOpType.mult
    ADD = mybir.AluOpType.add

    nc = bacc.Bacc()
    xT = nc.declare_dram_parameter("xT", [EMBED, N_SEQ], BF16, isOutput=False)
    # Q/K weights in 3 m-tiles of 128 cols: [Qh0|Qh1], [Kh0|Kh1], [Qh2|Kh2]
    wqk = nc.declare_dram_parameter("wqk", [EMBED, 384], BF16, isOutput=False)
    # V weights: [Vh0|Vh1|Vh2]
    wv = nc.declare_dram_parameter("wv", [EMBED, 192], BF16, isOutput=False)
    wp01 = nc.declare_dram_parameter("wp01", [128, EMBED], BF16, isOutput=False)
    wp2 = nc.declare_dram_parameter("wp2", [64, EMBED], BF16, isOutput=False)
    ones = nc.declare_dram_parameter("ones", [128, 3], BF16, isOutput=False)
    outT = nc.declare_dram_parameter("outT", [EMBED, N_SEQ], BF16, isOutput=True)

    with tile.TileContext(nc) as tc:
        with (
            tc.tile_pool(name="persist", bufs=1) as pp,
            tc.tile_pool(name="pt", bufs=28) as ptp,
            tc.tile_pool(name="work", bufs=6) as wk,
            tc.tile_pool(name="psS", bufs=2, space="PSUM") as psS,
            tc.tile_pool(name="psV", bufs=1, space="PSUM") as psV,
            tc.tile_pool(name="psP", bufs=2, space="PSUM") as psP,
        ):
            qk = [
                pp.tile([128, N_SEQ], BF16, tag=f"qk{m}", name=f"qk{m}")
                for m in range(3)
            ]
            qk2d = pp.tile([128, N_SEQ], BF16, tag="qk2d")
            vt = [
                pp.tile([128, 195], BF16, tag=f"vt{m}", name=f"vt{m}")
                for m in range(MT)
            ]
            xt = [
                pp.tile([128, N_SEQ], BF16, tag=f"xt{k}", name=f"xt{k}")
                for k in range(KT)
            ]
            wqk_t = [
                pp.tile([128, 384], BF16, tag=f"wqk{k}", name=f"wqkt{k}")
                for k in range(KT)
            ]
            wv_t = [
                pp.tile([128, 192], BF16, tag=f"wv{k}", name=f"wvt{k}")
                for k in range(KT)
            ]
            wp01_t = pp.tile([128, EMBED], BF16, tag="wp01", name="wp01t")
            wp2_t = pp.tile([64, EMBED], BF16, tag="wp2", name="wp2t")
            ao01 = pp.tile([128, N_SEQ], BF16, tag="ao01", name="ao01")
            ao2 = pp.tile([64, N_SEQ], BF16, tag="ao2", name="ao2")

            xT_ap = xT[:, :].rearrange("(t p) n -> t p n", p=128)
            wqk_ap = wqk[:, :].rearrange("(t p) n -> t p n", p=128)
            wv_ap = wv[:, :].rearrange("(t p) n -> t p n", p=128)
            for k in range(KT):
                nc.gpsimd.dma_start(out=wqk_t[k], in_=wqk_ap[k])
            for c in range(NCH):
                cs = slice(c * QCH, (c + 1) * QCH)
                for k in range(KT):
                    nc.gpsimd.dma_start(out=xt[k][:, cs], in_=xT_ap[k][:, cs])
            for k in range(KT):
                nc.gpsimd.dma_start(out=wv_t[k], in_=wv_ap[k])
            nc.gpsimd.dma_start(out=wp01_t, in_=wp01[:, :])
            nc.gpsimd.dma_start(out=wp2_t, in_=wp2[:, :])

            # ---- HAM warm-up: PE busy from ~1us (no DMA dependency) so the
            # clock gate is at K=8/8 when the real matmuls arrive.
            wsb = pp.tile([128, QCH], BF16, tag="wsb", name="wsb")
            nc.vector.memset(wsb, 0.0)
            warm = psP.tile([128, QCH], F32, tag="po", name="warm")
            for _ in range(30):
                nc.tensor.matmul(
                    warm, wsb[:, 0:128], wsb, start=True, stop=True,
                )

            # ---- QK^T m-tiles (K first so scores can start early) ----
            def qk_mtile(m, c2):
                cs = slice(c2 * 2 * QCH, (c2 + 1) * 2 * QCH)
                ps = psS.tile([128, 2 * QCH], F32, tag="sS", name="psqk")
                for k in range(KT):
                    nc.tensor.matmul(
                        ps[:, 0:QCH],
                        wqk_t[k][:, m * 128 : (m + 1) * 128],
                        xt[k][:, c2 * 2 * QCH : c2 * 2 * QCH + QCH],
                        start=(k == 0),
                        stop=(k == KT - 1),
                    )
                for k in range(KT):
                    nc.tensor.matmul(
                        ps[:, QCH:],
                        wqk_t[k][:, m * 128 : (m + 1) * 128],
                        xt[k][:, c2 * 2 * QCH + QCH : (c2 + 1) * 2 * QCH],
                        start=(k == 0),
                        stop=(k == KT - 1),
                    )
                nc.vector.tensor_copy(out=qk[m][:, cs], in_=ps)

            for m in (1, 0, 2):
                for c2 in range(NCH // 2):
                    qk_mtile(m, c2)
                if m == 2 and stages >= 2:
                    # [Qh2|Kh2] -> swapped copy [Kh2|Qh2]
                    nc.gpsimd.dma_start(out=qk2d[0:64, :], in_=qk[2][64:128, :])
                    nc.gpsimd.dma_start(out=qk2d[64:128, :], in_=qk[2][0:64, :])

            # ---- emission helpers ----
            def scores01_group(c, g):
                """Row-packed h0/h1 score pairs for 2 krow-tiles + exp."""
                qs = slice(c * QCH, (c + 1) * QCH)
                i0, i1 = 2 * g, 2 * g + 1
                s0 = psS.tile([128, 2 * QCH], F32, tag="sS", name="s0")
                s1 = psS.tile([128, 2 * QCH], F32, tag="sS", name="s1")
                for half, i in ((0, i0), (1, i1)):
                    ks = slice(i * 128, (i + 1) * 128)
                    hs = slice(half * QCH, (half + 1) * QCH)
                    nc.tensor.matmul(
                        s0[:, hs], qk[1][0:64, ks], qk[0][0:64, qs],
                        start=True, stop=True, tile_position=(0, 0),
                    )
                    nc.tensor.matmul(
                        s1[:, hs], qk[1][64:128, ks], qk[0][64:128, qs],
                        start=True, stop=True, tile_position=(64, 0),
                    )
                pt0 = ptp.tile([128, 2 * QCH], BF16, tag="ptg", name="pt0")
                pt1 = ptp.tile([128, 2 * QCH], BF16, tag="ptg", name="pt1")
                nc.scalar.activation(out=pt0, in_=s0, func=EXP, scale=0.125)
                nc.scalar.activation(out=pt1, in_=s1, func=EXP, scale=0.125)
                return pt0, pt1

            def attnv01_group(g, pt0, pt1, pv0, pv1):
                for half, i in ((0, 2 * g), (1, 2 * g + 1)):
                    hs = slice(half * QCH, (half + 1) * QCH)
                    nc.tensor.matmul(
                        pv0, vt[i][:, 0:65], pt0[:, hs],
                        start=(i == 0), stop=(i == MT - 1),
                    )
                    nc.tensor.matmul(
                        pv1, vt[i][:, 65:130], pt1[:, hs],
                        start=(i == 0), stop=(i == MT - 1),
                    )

            def scores2_group(c, g):
                qs = slice(c * QCH, (c + 1) * QCH)
                i0, i1 = 2 * g, 2 * g + 1
                ksA = slice(i0 * 128, (i0 + 1) * 128)
                ksB = slice(i1 * 128, (i1 + 1) * 128)
                s2 = psS.tile([128, 2 * QCH], F32, tag="sS", name="s2")
                nc.tensor.matmul(
                    s2[:, 0:QCH], qk2d[0:64, ksA], qk[2][0:64, qs],
                    start=True, stop=True, tile_position=(0, 0),
                )
                nc.tensor.matmul(
                    s2[:, QCH:], qk[2][64:128, ksB], qk2d[64:128, qs],
                    start=True, stop=True, tile_position=(64, 0),
                )
                pt2 = ptp.tile([128, 2 * QCH], BF16, tag="ptg", name="pt2")
                nc.scalar.activation(out=pt2, in_=s2, func=EXP, scale=0.125)
                return pt2

            def head2_pass(c, pv2, pre=None):
                for g in range(NG):
                    i0, i1 = 2 * g, 2 * g + 1
                    pt2 = pre[g] if pre is not None else scores2_group(c, g)
                    if stages >= 4:
                        nc.tensor.matmul(
                            pv2, vt[i0][:, 130:195], pt2[:, 0:QCH],
                            start=(i0 == 0), stop=False,
                        )
                        nc.tensor.matmul(
                            pv2, vt[i1][:, 130:195], pt2[:, QCH:],
                            start=False, stop=(i1 == MT - 1),
                        )

            def softmax_divide(h, pv, qs):
                """Drain pv once (frees the PSUM bank), then normalize."""
                ov = wk.tile([65, QCH], F32, tag="ov", name="ov")
                nc.vector.tensor_copy(out=ov, in_=pv)
                # reciprocal with all 128 lanes: reshape [1,512] -> [128,4]
                rw = wk.tile([128, QCH // 128], F32, tag="rw", name="rw")
                nc.sync.dma_start(out=rw, in_=ov[64:65, :])
                nc.vector.reciprocal(out=rw, in_=rw)
                rs0 = wk.tile([1, QCH], F32, tag="rs0", name="rs0")
                nc.sync.dma_start(out=rs0, in_=rw)
                bc = wk.tile([64, QCH], F32, tag="bc", name="bc")
                nc.gpsimd.partition_broadcast(bc, rs0)
                if h == 0:
                    dst = ao01[0:64, qs]
                elif h == 1:
                    dst = ao01[64:128, qs]
                else:
                    dst = ao2[:, qs]
                nc.vector.tensor_mul(out=dst, in0=ov[0:64, :], in1=bc)

            def proj_chunk(c):
                qs = slice(c * QCH, (c + 1) * QCH)
                for m in range(KT):
                    ms = slice(m * 128, (m + 1) * 128)
                    po = psP.tile([128, QCH], F32, tag="po", name="po")
                    nc.tensor.matmul(po, wp01_t[:, ms], ao01[:, qs], start=True, stop=False)
                    nc.tensor.matmul(po, wp2_t[:, ms], ao2[:, qs], start=False, stop=True)
                    ot = wk.tile([128, QCH], BF16, tag="ot", name="ot")
                    nc.vector.tensor_copy(out=ot, in_=po)
                    nc.sync.dma_start(
                        out=outT[:, :].rearrange("(t p) n -> t p n", p=128)[m][:, qs],
                        in_=ot,
                    )

            # ---- chunk-0 h0/h1 scores interleaved with V-natural ----
            def vnat_mtile(m):
                ps = psP.tile([128, 192], F32, tag="po", name="psv")
                for k in range(KT):
                    nc.tensor.matmul(
                        ps,
                        xt[k][:, m * 128 : (m + 1) * 128],
                        wv_t[k],
                        start=(k == 0),
                        stop=(k == KT - 1),
                    )
                for h in range(N_HEADS_CORE):
                    nc.vector.tensor_copy(
                        out=vt[m][:, 65 * h : 65 * h + 64],
                        in_=ps[:, 64 * h : 64 * h + 64],
                    )
                nc.gpsimd.dma_start(
                    out=vt[m].rearrange("p (h c) -> p h c", c=65)[:, :, 64],
                    in_=ones[:, :],
                )

            def proj_mtile(c, m):
                qs = slice(c * QCH, (c + 1) * QCH)
                ms = slice(m * 128, (m + 1) * 128)
                po = psP.tile([128, QCH], F32, tag="po", name="po")
                nc.tensor.matmul(po, wp01_t[:, ms], ao01[:, qs], start=True, stop=False)
                nc.tensor.matmul(po, wp2_t[:, ms], ao2[:, qs], start=False, stop=True)
                ot = wk.tile([128, QCH], BF16, tag="ot", name="ot")
                nc.vector.tensor_copy(out=ot, in_=po)
                nc.sync.dma_start(
                    out=outT[:, :].rearrange("(t p) n -> t p n", p=128)[m][:, qs],
                    in_=ot,
                )

            # pt rings held across the pipeline
            pt01 = {}
            pt2 = {}
            if stages >= 3:
                for g in range(NG):
                    pt01[(0, g)] = scores01_group(0, g)
                    vnat_mtile(2 * g)
                    vnat_mtile(2 * g + 1)

                # ---- software-pipelined chunk stream ----
                for c in range(NCH):
                    qs = slice(c * QCH, (c + 1) * QCH)
                    pv0 = psV.tile([65, QCH], F32, tag="pv0", name="pv0")
                    pv1 = psV.tile([65, QCH], F32, tag="pv1", name="pv1")
                    for g in range(NG):
                        if stages >= 4:
                            p0, p1 = pt01.pop((c, g))
                            attnv01_group(g, p0, p1, pv0, pv1)
                        if stages >= 5 and c >= 1 and g < KT:
                            proj_mtile(c - 1, g)
                        pt2[(c, g)] = scores2_group(c, g)
                    if stages >= 4:
                        softmax_divide(0, pv0, qs)
                        softmax_divide(1, pv1, qs)

                    pv2 = psV.tile([65, QCH], F32, tag="pv0", name="pv2")
                    for g in range(NG):
                        i0, i1 = 2 * g, 2 * g + 1
                        p2 = pt2.pop((c, g))
                        if stages >= 4:
                            nc.tensor.matmul(
                                pv2, vt[i0][:, 130:195], p2[:, 0:QCH],
                                start=(i0 == 0), stop=False,
                            )
                            nc.tensor.matmul(
                                pv2, vt[i1][:, 130:195], p2[:, QCH:],
                                start=False, stop=(i1 == MT - 1),
                            )
                        if c < NCH - 1:
                            pt01[(c + 1, g)] = scores01_group(c + 1, g)
                    if stages >= 4:
                        softmax_divide(2, pv2, qs)

                # keep the PE clock gate open across the divide chain that
                # precedes the epilogue projection (psS is idle by now)
                tailw = psS.tile([128, QCH], F32, tag="sS", name="tailw")
                for _ in range(8):
                    nc.tensor.matmul(
                        tailw, wsb[:, 0:128], wsb, start=True, stop=True,
                    )

                # epilogue: last chunk's projection
                if stages >= 5:
                    for m in range(KT):
                        proj_mtile(NCH - 1, m)

    nc.compile()
    return nc


def _get_nc():
    if "nc" not in _CACHED:
        _CACHED["nc"] = _build()
    return _CACHED["nc"]


def _shard_inputs(x, w_qkv, w_proj):
    """Build the 8 per-core input maps (bf16 operands)."""
    import ml_dtypes

    bf = ml_dtypes.bfloat16
    in_maps = []
    for core in range(N_CORES):
        b = core // 4
        h0 = 3 * (core % 4)
        heads = [h0, h0 + 1, h0 + 2]
        xTc = np.ascontiguousarray(x[b].T).astype(bf)
        wq = [w_qkv[:, h * HD : (h + 1) * HD] for h in heads]
        wk_ = [w_qkv[:, EMBED + h * HD : EMBED + (h + 1) * HD] for h in heads]
        wv_ = [
            w_qkv[:, 2 * EMBED + h * HD : 2 * EMBED + (h + 1) * HD] for h in heads
        ]
        wqk = np.concatenate(
            [wq[0], wq[1], wk_[0], wk_[1], wq[2], wk_[2]], axis=1
        ).astype(bf)
        wvp = np.concatenate([wv_[0], wv_[1], wv_[2]], axis=1).astype(bf)
        wps = [
            np.ascontiguousarray(w_proj[h * HD : (h + 1) * HD, :]).astype(bf)
            for h in heads
        ]
        in_maps.append(
            {
                "ones": np.ones((128, 3), bf),
                "xT": xTc,
                "wqk": np.ascontiguousarray(wqk),
                "wv": np.ascontiguousarray(wvp),
                "wp01": np.ascontiguousarray(np.concatenate([wps[0], wps[1]], axis=0)),
                "wp2": wps[2],
            }
        )
    return in_maps


def kernel(x, w_qkv, w_proj, _trace=False):
    from concourse.bass_utils import run_bass_kernel_spmd

    x = np.asarray(x, dtype=np.float32)
    w_qkv = np.asarray(w_qkv, dtype=np.float32)
    w_proj = np.asarray(w_proj, dtype=np.float32)

    nc = _get_nc()
    in_maps = _shard_inputs(x, w_qkv, w_proj)
    res = run_bass_kernel_spmd(
        nc, in_maps, core_ids=list(range(N_CORES)), trace=_trace
    )
    _CACHED["last_results"] = res

    out = np.empty((2, N_SEQ, EMBED), dtype=np.float32)
    for b in range(2):
        acc = res.results[4 * b]["outT"].astype(np.float32).copy()
        for g in range(1, 4):
            acc += res.results[4 * b + g]["outT"].astype(np.float32)
        out[b] = acc.T
    return out

